# revision 27
# baseline (speedup 1.0000x reference)
"""CPSF fused codebook kernel for Trainium2 (8 NeuronCores, codebook-parallel).

v2: erf reformulation. The K=8 Gauss-Legendre sum equals (to ~1e-12) the
exact ray integral, so per (b,m):

  wgt = alpha*sqrt(pi/(4|G|)) * exp(base) * [erf(sqG*(1-x)) + erf(sqG*x)]
  base = c_o*(q0 + dist_d) - c_o*x^2 + Gd*y^2      (x,y = Re/Im <dj, z-zj>)

exp(base) is evaluated as Sigmoid(base) (identical for base << 0, which
holds for every contributing entry) so Erf/Square/Sigmoid share one ACT
table set. x/y/f3z/f3d matmuls run in bf16 (host-validated: no error
change vs fp32), f3c stays float32r (c_o rounding vs large |z|^2 terms).
F3 accumulates in PSUM via PE; the squares and base combine run on
ACT/DVE/gpsimd; square cross-terms fold into f3z on the host and
alpha*sqrt(pi/4|G|) folds into That.

Sharding: M (4096) split 8 ways -> 512 entries (4 tiles of 128) per core;
each core sees all B=2048 queries; host sums the 8 partial [B,S] outputs.
"""

import numpy as np
import ml_dtypes

B, M, N, S, K = 2048, 4096, 64, 64, 8
EPS = 1e-3
NCORES = 8
ML = M // NCORES          # 512 codebook entries per core
MT = ML // 128            # 4 m-tiles per core
NH = 2                    # b-halves per tile for PSUM staging
HB = B // NH              # 1024
f32 = np.float32

_CACHE = {}


def _prep(z_re, z_im, d_re, d_im, zj_re, zj_im, dj_re, dj_im,
          That_re, That_im, alpha, sig_par, sig_perp):
    """Host-side packing: fp64 exact, cast to fp32 at the end."""
    x64 = lambda a: np.asarray(a, np.float64)
    zr, zi, dr, di = map(x64, (z_re, z_im, d_re, d_im))
    zjr, zji, djr, dji = map(x64, (zj_re, zj_im, dj_re, dj_im))

    sp2 = x64(sig_par)**2 + EPS
    so2 = x64(sig_perp)**2 + EPS
    G = -0.5 / sp2
    c_o = -0.5 / so2
    Gd = G - c_o
    sgn = np.where(Gd >= 0, 1.0, -1.0)
    sqG = np.sqrt(-G)
    sq_x = np.sqrt(-c_o)
    sq_y = np.sqrt(np.abs(Gd))

    c_re = (djr * zjr + dji * zji).sum(-1)       # Re<conj(dj), zj>
    c_im = (djr * zji - dji * zjr).sum(-1)
    nzj = (zjr**2 + zji**2).sum(-1)
    nz = (zr**2 + zi**2).sum(-1)                 # [B]
    nd = (dr**2 + di**2).sum(-1)
    muz, mud = nz.mean(), nd.mean()

    bx = -sq_x * c_re                            # xx = (xps + bx)^2
    by = -sq_y * c_im                            # yy = sgn*(yps + by)^2

    djx = np.concatenate([djr.T, dji.T], 0) * sq_x[None, :]    # [128, M]
    djy = np.concatenate([-dji.T, djr.T], 0) * sq_y[None, :]
    # f3z: -2c_o*zj contraction + the linear cross-terms of both squares
    f3z = ((-2.0 * c_o) * np.concatenate([zjr.T, zji.T], 0)
           + (2.0 * bx) * djx + (2.0 * sgn * by) * djy)
    f3d = (-2.0 * c_o) * np.concatenate([djr.T, dji.T], 0)
    f3c = np.stack([c_o, c_o])                               # [2, M]
    const0 = c_o * (nzj + 1.0 + muz + mud) + bx * bx + sgn * by * by
    rhsc = np.stack([nz - muz, nd - mud])                    # [2, B]

    # That payload with alpha*sqrt(pi/4|G|) folded in, bf16 on device
    scl = x64(alpha) * np.sqrt(np.pi / (4.0 * (-G)))
    that2 = np.concatenate([x64(That_re), x64(That_im)], 1) * scl[:, None]

    # pk: per m-tile block [djx | djy | f3z], 128 cols each; f3d ships bf16
    nt = M // 128
    pk = np.empty((128, nt * 384), np.float64)
    for j in range(nt):
        ms = slice(j * 128, (j + 1) * 128)
        pk[:, j * 384 + 0:j * 384 + 128] = djx[:, ms]
        pk[:, j * 384 + 128:j * 384 + 256] = djy[:, ms]
        pk[:, j * 384 + 256:j * 384 + 384] = f3z[:, ms]

    # per-partition params, [128, nt] each
    rs = lambda a: a.reshape(nt, 128).T
    c = lambda a: np.ascontiguousarray(a, dtype=f32)
    return dict(pk=c(pk), f3c=c(f3c), rhsc=c(rhsc),
                zst=c(np.concatenate([zr.T, zi.T], 0)),
                dst=c(np.concatenate([dr.T, di.T], 0)),
                f3d=c(f3d), that2=c(that2),
                s1=c(rs(-sqG / sq_x)), b1=c(rs(sqG * (1.0 + c_re))),
                s2=c(rs(sqG / sq_x)), b2=c(rs(-sqG * c_re)),
                c0=c(rs(const0)), sg=c(rs(sgn)))


def _core_slices(p, cid):
    """Per-core in_map from the full packed arrays (m-sharded)."""
    jt = slice(cid * MT * 384, (cid + 1) * MT * 384)         # pk cols
    ms = slice(cid * ML, (cid + 1) * ML)
    jc = slice(cid * MT, (cid + 1) * MT)
    cc = np.ascontiguousarray
    return {"pk": cc(p["pk"][:, jt]), "f3c": cc(p["f3c"][:, ms]),
            "f3d": cc(p["f3d"][:, ms]),
            "that2": cc(p["that2"][ms, :]),
            "zst": p["zst"], "dst": p["dst"], "rhsc": p["rhsc"],
            "params": cc(np.concatenate(
                [p[k][:, jc] for k in ("s1", "b1", "s2", "b2", "c0", "sg")],
                axis=1))}


def _device_maps(maps):
    dev_maps = []
    for m in maps:
        dm = dict(m)
        dm["that2"] = np.ascontiguousarray(
            m["that2"].astype(ml_dtypes.bfloat16))
        dm["f3db"] = np.ascontiguousarray(
            m["f3d"].astype(ml_dtypes.bfloat16))
        dm["dstb"] = np.ascontiguousarray(
            m["dst"].astype(ml_dtypes.bfloat16))
        dm["zst"] = np.ascontiguousarray(
            m["zst"].astype(ml_dtypes.bfloat16))
        dm["pk"] = np.ascontiguousarray(
            m["pk"].astype(ml_dtypes.bfloat16))
        del dm["f3d"], dm["dst"]
        dev_maps.append(dm)
    return dev_maps


def _bf16(a):
    return np.asarray(a, f32).astype(ml_dtypes.bfloat16).astype(f32)


def _emulate_core(m):
    """Numpy emulation of one core's device program (fp32-faithful-ish)."""
    from scipy.special import erf as erf64
    zst, rhsc = m["zst"].astype(f32), m["rhsc"]
    dst = m["dstb"].astype(f32)
    f3db = m["f3db"].astype(f32)
    prm = m["params"]
    s1, b1 = prm[:, 0:MT], prm[:, MT:2 * MT]
    s2, b2 = prm[:, 2 * MT:3 * MT], prm[:, 3 * MT:4 * MT]
    c0, sg = prm[:, 4 * MT:5 * MT], prm[:, 5 * MT:6 * MT]
    that_bf = m["that2"].astype(f32)             # [ML, 128]
    t_acc = np.zeros((128, B), f32)
    for j in range(MT):
        pkj = m["pk"][:, j * 384:(j + 1) * 384].astype(f32)
        djx_t, djy_t = pkj[:, 0:128], pkj[:, 128:256]
        f3z_t = pkj[:, 256:384]
        f3d_t = f3db[:, j * 128:(j + 1) * 128]
        xps = (djx_t.T @ zst).astype(f32)
        yps = (djy_t.T @ zst).astype(f32)
        base = (f3z_t.T @ zst + f3d_t.T @ dst
                + m["f3c"][:, j * 128:(j + 1) * 128].T @ rhsc).astype(f32)
        xx = np.square(xps, dtype=f32)
        yy = (sg[:, j:j + 1] * np.square(yps)).astype(f32)
        base = (base + xx + yy).astype(f32)
        e1 = erf64(np.float64(1) * s1[:, j:j + 1] * xps
                   + b1[:, j:j + 1]).astype(f32)
        e2 = erf64(np.float64(1) * s2[:, j:j + 1] * xps
                   + b2[:, j:j + 1]).astype(f32)
        h = (e1 + e2).astype(f32)
        a64 = (base + c0[:, j:j + 1]).astype(f32).astype(np.float64)
        EB = (np.exp(a64) / (1.0 + np.exp(a64))).astype(f32)
        wgt = _bf16((_bf16(EB) * _bf16(h)).astype(f32))
        that_t = that_bf[j * 128:(j + 1) * 128, :]
        t_acc += (that_t.T @ wgt).astype(f32)
    return t_acc


def _build_bass():
    import concourse.bacc as bacc
    import concourse.mybir as mybir
    from concourse import tile

    dt = mybir.dt.float32
    bfdt = mybir.dt.bfloat16
    f32r = mybir.dt.float32r
    AF = mybir.ActivationFunctionType
    AO = mybir.AluOpType
    nc = bacc.Bacc("TRN2", target_bir_lowering=False, debug=False)

    dram = {}
    for name, shape, dty in [("zst", [128, B], bfdt),
                             ("dstb", [128, B], bfdt),
                             ("rhsc", [2, B], f32r),
                             ("pk", [128, MT * 384], bfdt),
                             ("f3db", [128, ML], bfdt),
                             ("f3c", [2, ML], f32r),
                             ("that2", [ML, 128], bfdt),
                             ("params", [128, 6 * MT], dt)]:
        dram[name] = nc.dram_tensor(name, shape, dty, kind="ExternalInput")
    tout = nc.dram_tensor("tout", [128, B], dt, kind="ExternalOutput")

    with tile.TileContext(nc) as tc:
        with tc.tile_pool(name="const", bufs=1) as cpool:
            params = cpool.tile([128, 6 * MT], dt)
            s1 = params[:, 0:MT]
            b1 = params[:, MT:2 * MT]
            s2 = params[:, 2 * MT:3 * MT]
            b2 = params[:, 3 * MT:4 * MT]
            c0 = params[:, 4 * MT:5 * MT]
            sg = params[:, 5 * MT:6 * MT]
            zst = cpool.tile([128, B], bfdt)
            dstb = cpool.tile([128, B], bfdt)
            rhsc = cpool.tile([2, B], f32r)
            that_all = cpool.tile([128, MT * 128], bfdt)
            f3db_all = cpool.tile([128, ML], bfdt)
            f3c_all = cpool.tile([2, ML], f32r)
            warm = cpool.tile([128, 8], dt)

            nc.sync.dma_start(params[:, :], dram["params"][:, :])
            # fire the ACT erf/sigmoid table-load ASAP, overlap with DMAs
            nc.scalar.activation(warm[:, :], params[:, 0:8], AF.Erf)

            with (
                tc.tile_pool(name="lhs", bufs=4) as lpool,
                tc.tile_pool(name="work", bufs=2) as wpool,
                tc.tile_pool(name="etile", bufs=2) as epool,
                tc.tile_pool(name="wgtp", bufs=1) as gpool,
            ):
                pks = [lpool.tile([128, 384], bfdt, tag="pk",
                                  name=f"pk{j}") for j in range(MT)]
                QB = B // 4
                nc.sync.dma_start(pks[0][:, :], dram["pk"][:, 0:384])
                nc.scalar.dma_start(zst[:, 0:QB], dram["zst"][:, 0:QB])
                nc.gpsimd.dma_start(zst[:, QB:2 * QB],
                                    dram["zst"][:, QB:2 * QB])
                nc.scalar.dma_start(zst[:, 2 * QB:3 * QB],
                                    dram["zst"][:, 2 * QB:3 * QB])
                nc.sync.dma_start(zst[:, 3 * QB:B],
                                  dram["zst"][:, 3 * QB:B])
                nc.scalar.dma_start(rhsc[:, :], dram["rhsc"][:, :])
                nc.sync.dma_start(dstb[:, 0:HB], dram["dstb"][:, 0:HB])
                nc.scalar.dma_start(dstb[:, HB:B], dram["dstb"][:, HB:B])
                nc.scalar.dma_start(f3c_all[:, :], dram["f3c"][:, :])
                nc.sync.dma_start(f3db_all[:, :], dram["f3db"][:, :])
                for j in range(1, MT):
                    nc.sync.dma_start(pks[j][:, :],
                                      dram["pk"][:, j * 384:(j + 1) * 384])
                nc.sync.dma_start(
                    that_all[:, :].rearrange("p (j c) -> p j c", j=MT),
                    dram["that2"][:, :].rearrange("(j p) c -> p j c", p=128))

                wgts = []
                with (
                    tc.tile_pool(name="xps", bufs=1, space="PSUM") as xpool,
                    tc.tile_pool(name="yps", bufs=1, space="PSUM") as ypool,
                    tc.tile_pool(name="bps", bufs=2, space="PSUM") as bpool,
                ):
                    pending = None

                    def finalize(st):
                        bs2_p, EB_p, wgt_p, hh_p, hs_p, j_p = st
                        nc.scalar.activation(EB_p[:, :], bs2_p[:, :],
                                             AF.Sigmoid,
                                             bias=c0[:, j_p:j_p + 1])
                        nc.vector.tensor_mul(wgt_p[:, hs_p], EB_p[:, :],
                                             hh_p[:, hs_p])

                    for j in range(MT):
                        pk_t = pks[j]
                        djx_t = pk_t[:, 0:128]
                        djy_t = pk_t[:, 128:256]
                        f3z_t = pk_t[:, 256:384]
                        f3db_t = f3db_all[:, j * 128:(j + 1) * 128]
                        f3c_t = f3c_all[:, j * 128:(j + 1) * 128]

                        wgt = gpool.tile([128, B], bfdt, tag=f"wgt{j}")
                        wgts.append(wgt)
                        # per-tile full-B e1/e2 so h is one wide op
                        e1 = epool.tile([128, B], dt, tag="e1")
                        e2 = epool.tile([128, B], dt, tag="e2")
                        hh = epool.tile([128, B], bfdt, tag="hh")
                        for h in range(NH):
                            hs = slice(h * HB, (h + 1) * HB)
                            x_ps = xpool.tile([128, HB], dt, tag="x",
                                              name=f"x{j}_{h}")
                            y_ps = ypool.tile([128, HB], dt, tag="y",
                                              name=f"y{j}_{h}")
                            b_ps = bpool.tile([128, HB], dt, tag="b",
                                              name=f"b{j}_{h}")
                            xx = wpool.tile([128, HB], dt, tag="xx")
                            ysb = wpool.tile([128, HB], dt, tag="ysb")
                            ysq = wpool.tile([128, HB], dt, tag="ysq")
                            bs1 = wpool.tile([128, HB], dt, tag="bs1")
                            bs2 = wpool.tile([128, HB], dt, tag="bs2")
                            EB = wpool.tile([128, HB], bfdt, tag="EB")

                            for q in range(2):
                                qs = slice(h * HB + q * 512,
                                           h * HB + (q + 1) * 512)
                                qo = slice(q * 512, (q + 1) * 512)
                                nc.tensor.matmul(x_ps[:, qo], djx_t,
                                                 zst[:, qs],
                                                 start=True, stop=True)
                            for q in range(2):
                                qs = slice(h * HB + q * 512,
                                           h * HB + (q + 1) * 512)
                                qo = slice(q * 512, (q + 1) * 512)
                                nc.tensor.matmul(y_ps[:, qo], djy_t,
                                                 zst[:, qs],
                                                 start=True, stop=True)
                            nc.vector.tensor_copy(ysb[:, :], y_ps[:, :])
                            nc.gpsimd.tensor_mul(ysq[:, :], ysb[:, :],
                                                 ysb[:, :])
                            # ACT erf pair + x-square off x_ps
                            nc.scalar.activation(e1[:, hs], x_ps[:, :],
                                                 AF.Erf,
                                                 bias=b1[:, j:j + 1],
                                                 scale=s1[:, j:j + 1])
                            nc.scalar.activation(e2[:, hs], x_ps[:, :],
                                                 AF.Erf,
                                                 bias=b2[:, j:j + 1],
                                                 scale=s2[:, j:j + 1])
                            nc.scalar.activation(xx[:, :], x_ps[:, :],
                                                 AF.Square)
                            nc.gpsimd.tensor_add(hh[:, hs], e1[:, hs],
                                                 e2[:, hs])
                            # base accumulation in PSUM via PE
                            for q in range(2):
                                qs = slice(h * HB + q * 512,
                                           h * HB + (q + 1) * 512)
                                qo = slice(q * 512, (q + 1) * 512)
                                nc.tensor.matmul(b_ps[:, qo], f3z_t,
                                                 zst[:, qs],
                                                 start=True, stop=False)
                                nc.tensor.matmul(b_ps[:, qo], f3db_t,
                                                 dstb[:, qs],
                                                 start=False, stop=False)
                                nc.tensor.matmul(b_ps[:, qo], f3c_t,
                                                 rhsc[:, qs],
                                                 start=False, stop=True)
                            # base = (b_ps + xx) + sgn*ysq on DVE
                            nc.vector.tensor_add(bs1[:, :], xx[:, :],
                                                 b_ps[:, :])
                            nc.vector.scalar_tensor_tensor(
                                bs2[:, :], ysq[:, :], sg[:, j:j + 1],
                                bs1[:, :], AO.mult, AO.add)
                            if pending is not None:
                                finalize(pending)
                            pending = (bs2, EB, wgt, hh, hs, j)
                    finalize(pending)

                # payload matmuls: T = sum_j that_j^T wgt_j
                with tc.tile_pool(name="tp", bufs=1, space="PSUM") as tpool:
                    ocp = wpool.tile([128, B], dt, tag="ocp")
                    tph = tpool.tile([128, B], dt, tag="tph")
                    for j in range(MT):
                        that_j = that_all[:, j * 128:(j + 1) * 128]
                        for q in range(4):
                            qo = slice(q * 512, (q + 1) * 512)
                            nc.tensor.matmul(tph[:, qo], that_j,
                                             wgts[j][:, qo],
                                             start=(j == 0),
                                             stop=(j == MT - 1))
                            if j == MT - 1:
                                nc.scalar.copy(ocp[:, qo], tph[:, qo])
                                nc.sync.dma_start(tout[:, qo], ocp[:, qo])

    nc.compile()
    return nc


def kernel(z_re, z_im, d_re, d_im, zj_re, zj_im, dj_re, dj_im,
           That_re, That_im, alpha, sig_par, sig_perp, _emulate=False):
    p = _prep(z_re, z_im, d_re, d_im, zj_re, zj_im, dj_re, dj_im,
              That_re, That_im, alpha, sig_par, sig_perp)
    maps = [_core_slices(p, c) for c in range(NCORES)]
    dev_maps = _device_maps(maps)

    if _emulate:
        outs = [_emulate_core(m) for m in dev_maps]
    else:
        from concourse.bass_utils import run_bass_kernel_spmd
        if "nc" not in _CACHE:
            _CACHE["nc"] = _build_bass()
        res = run_bass_kernel_spmd(_CACHE["nc"], dev_maps,
                                   core_ids=list(range(NCORES)))
        outs = [res.results[c]["tout"] for c in range(NCORES)]

    full = np.zeros((128, B), np.float64)
    for o in outs:
        full += o.astype(np.float64)
    full = full.astype(f32).T                   # [B, 128]
    return (full[:, :S] + 1j * full[:, S:]).astype(np.complex64)


# revision 29
# speedup vs baseline: 1.0170x; 1.0170x over previous
"""CPSF fused codebook kernel for Trainium2 (8 NeuronCores, codebook-parallel).

v2: erf reformulation. The K=8 Gauss-Legendre sum equals (to ~1e-12) the
exact ray integral, so per (b,m):

  wgt = alpha*sqrt(pi/(4|G|)) * exp(base) * [erf(sqG*(1-x)) + erf(sqG*x)]
  base = c_o*(q0 + dist_d) - c_o*x^2 + Gd*y^2      (x,y = Re/Im <dj, z-zj>)

exp(base) is evaluated as Sigmoid(base) (identical for base << 0, which
holds for every contributing entry) so Erf/Square/Sigmoid share one ACT
table set. x/y/f3z/f3d matmuls run in bf16 (host-validated: no error
change vs fp32), f3c stays float32r (c_o rounding vs large |z|^2 terms).
F3 accumulates in PSUM via PE; the squares and base combine run on
ACT/DVE/gpsimd; square cross-terms fold into f3z on the host and
alpha*sqrt(pi/4|G|) folds into That.

Sharding: M (4096) split 8 ways -> 512 entries (4 tiles of 128) per core;
each core sees all B=2048 queries; host sums the 8 partial [B,S] outputs.
"""

import numpy as np
import ml_dtypes

B, M, N, S, K = 2048, 4096, 64, 64, 8
EPS = 1e-3
NCORES = 8
ML = M // NCORES          # 512 codebook entries per core
MT = ML // 128            # 4 m-tiles per core
NH = 2                    # b-halves per tile for PSUM staging
HB = B // NH              # 1024
f32 = np.float32

_CACHE = {}


def _prep(z_re, z_im, d_re, d_im, zj_re, zj_im, dj_re, dj_im,
          That_re, That_im, alpha, sig_par, sig_perp):
    """Host-side packing: fp64 exact, cast to fp32 at the end."""
    x64 = lambda a: np.asarray(a, np.float64)
    zr, zi, dr, di = map(x64, (z_re, z_im, d_re, d_im))
    zjr, zji, djr, dji = map(x64, (zj_re, zj_im, dj_re, dj_im))

    sp2 = x64(sig_par)**2 + EPS
    so2 = x64(sig_perp)**2 + EPS
    G = -0.5 / sp2
    c_o = -0.5 / so2
    Gd = G - c_o
    sgn = np.where(Gd >= 0, 1.0, -1.0)
    sqG = np.sqrt(-G)
    sq_x = np.sqrt(-c_o)
    sq_y = np.sqrt(np.abs(Gd))

    c_re = (djr * zjr + dji * zji).sum(-1)       # Re<conj(dj), zj>
    c_im = (djr * zji - dji * zjr).sum(-1)
    nzj = (zjr**2 + zji**2).sum(-1)
    nz = (zr**2 + zi**2).sum(-1)                 # [B]
    nd = (dr**2 + di**2).sum(-1)
    muz, mud = nz.mean(), nd.mean()

    bx = -sq_x * c_re                            # xx = (xps + bx)^2
    by = -sq_y * c_im                            # yy = sgn*(yps + by)^2

    djx = np.concatenate([djr.T, dji.T], 0) * sq_x[None, :]    # [128, M]
    djy = np.concatenate([-dji.T, djr.T], 0) * sq_y[None, :]
    # f3z: -2c_o*zj contraction + the linear cross-terms of both squares
    f3z = ((-2.0 * c_o) * np.concatenate([zjr.T, zji.T], 0)
           + (2.0 * bx) * djx + (2.0 * sgn * by) * djy)
    f3d = (-2.0 * c_o) * np.concatenate([djr.T, dji.T], 0)
    f3c = np.stack([c_o, c_o])                               # [2, M]
    const0 = c_o * (nzj + 1.0 + muz + mud) + bx * bx + sgn * by * by
    rhsc = np.stack([nz - muz, nd - mud])                    # [2, B]

    # That payload with alpha*sqrt(pi/4|G|) folded in, bf16 on device
    scl = x64(alpha) * np.sqrt(np.pi / (4.0 * (-G)))
    that2 = np.concatenate([x64(That_re), x64(That_im)], 1) * scl[:, None]

    # pk: per m-tile block [djx | djy | f3z], 128 cols each; f3d ships bf16
    nt = M // 128
    pk = np.empty((128, nt * 384), np.float64)
    for j in range(nt):
        ms = slice(j * 128, (j + 1) * 128)
        pk[:, j * 384 + 0:j * 384 + 128] = djx[:, ms]
        pk[:, j * 384 + 128:j * 384 + 256] = djy[:, ms]
        pk[:, j * 384 + 256:j * 384 + 384] = f3z[:, ms]

    # per-partition params, [128, nt] each
    rs = lambda a: a.reshape(nt, 128).T
    c = lambda a: np.ascontiguousarray(a, dtype=f32)
    return dict(pk=c(pk), f3c=c(f3c), rhsc=c(rhsc),
                zst=c(np.concatenate([zr.T, zi.T], 0)),
                dst=c(np.concatenate([dr.T, di.T], 0)),
                f3d=c(f3d), that2=c(that2),
                s1=c(rs(-sqG / sq_x)), b1=c(rs(sqG * (1.0 + c_re))),
                s2=c(rs(sqG / sq_x)), b2=c(rs(-sqG * c_re)),
                c0=c(rs(const0)), sg=c(rs(sgn)))


def _core_slices(p, cid):
    """Per-core in_map from the full packed arrays (m-sharded)."""
    jt = slice(cid * MT * 384, (cid + 1) * MT * 384)         # pk cols
    ms = slice(cid * ML, (cid + 1) * ML)
    jc = slice(cid * MT, (cid + 1) * MT)
    cc = np.ascontiguousarray
    return {"pk": cc(p["pk"][:, jt]), "f3c": cc(p["f3c"][:, ms]),
            "f3d": cc(p["f3d"][:, ms]),
            "that2": cc(p["that2"][ms, :]),
            "zst": p["zst"], "dst": p["dst"], "rhsc": p["rhsc"],
            "params": cc(np.concatenate(
                [p[k][:, jc] for k in ("s1", "b1", "s2", "b2", "c0", "sg")],
                axis=1))}


def _device_maps(maps):
    dev_maps = []
    for m in maps:
        dm = dict(m)
        dm["that2"] = np.ascontiguousarray(
            m["that2"].astype(ml_dtypes.bfloat16))
        dm["f3db"] = np.ascontiguousarray(
            m["f3d"].astype(ml_dtypes.bfloat16))
        dm["dstb"] = np.ascontiguousarray(
            m["dst"].astype(ml_dtypes.bfloat16))
        dm["zst"] = np.ascontiguousarray(
            m["zst"].astype(ml_dtypes.bfloat16))
        dm["pk"] = np.ascontiguousarray(
            m["pk"].astype(ml_dtypes.bfloat16))
        del dm["f3d"], dm["dst"]
        dev_maps.append(dm)
    return dev_maps


def _bf16(a):
    return np.asarray(a, f32).astype(ml_dtypes.bfloat16).astype(f32)


def _emulate_core(m):
    """Numpy emulation of one core's device program (fp32-faithful-ish)."""
    from scipy.special import erf as erf64
    zst, rhsc = m["zst"].astype(f32), m["rhsc"]
    dst = m["dstb"].astype(f32)
    f3db = m["f3db"].astype(f32)
    prm = m["params"]
    s1, b1 = prm[:, 0:MT], prm[:, MT:2 * MT]
    s2, b2 = prm[:, 2 * MT:3 * MT], prm[:, 3 * MT:4 * MT]
    c0, sg = prm[:, 4 * MT:5 * MT], prm[:, 5 * MT:6 * MT]
    that_bf = m["that2"].astype(f32)             # [ML, 128]
    t_acc = np.zeros((128, B), f32)
    for j in range(MT):
        pkj = m["pk"][:, j * 384:(j + 1) * 384].astype(f32)
        djx_t, djy_t = pkj[:, 0:128], pkj[:, 128:256]
        f3z_t = pkj[:, 256:384]
        f3d_t = f3db[:, j * 128:(j + 1) * 128]
        xps = (djx_t.T @ zst).astype(f32)
        yps = (djy_t.T @ zst).astype(f32)
        base = (f3z_t.T @ zst + f3d_t.T @ dst
                + m["f3c"][:, j * 128:(j + 1) * 128].T @ rhsc).astype(f32)
        xx = np.square(xps, dtype=f32)
        yy = (sg[:, j:j + 1] * np.square(yps)).astype(f32)
        base = (base + xx + yy).astype(f32)
        e1 = erf64(np.float64(1) * s1[:, j:j + 1] * xps
                   + b1[:, j:j + 1]).astype(f32)
        e2 = erf64(np.float64(1) * s2[:, j:j + 1] * xps
                   + b2[:, j:j + 1]).astype(f32)
        h = (e1 + e2).astype(f32)
        a64 = (base + c0[:, j:j + 1]).astype(f32).astype(np.float64)
        EB = (np.exp(a64) / (1.0 + np.exp(a64))).astype(f32)
        wgt = _bf16((_bf16(EB) * _bf16(h)).astype(f32))
        that_t = that_bf[j * 128:(j + 1) * 128, :]
        t_acc += (that_t.T @ wgt).astype(f32)
    return t_acc


def _build_bass():
    import concourse.bacc as bacc
    import concourse.mybir as mybir
    from concourse import tile

    dt = mybir.dt.float32
    bfdt = mybir.dt.bfloat16
    f32r = mybir.dt.float32r
    AF = mybir.ActivationFunctionType
    AO = mybir.AluOpType
    nc = bacc.Bacc("TRN2", target_bir_lowering=False, debug=False)

    dram = {}
    for name, shape, dty in [("zst", [128, B], bfdt),
                             ("dstb", [128, B], bfdt),
                             ("rhsc", [2, B], f32r),
                             ("pk", [128, MT * 384], bfdt),
                             ("f3db", [128, ML], bfdt),
                             ("f3c", [2, ML], f32r),
                             ("that2", [ML, 128], bfdt),
                             ("params", [128, 6 * MT], dt)]:
        dram[name] = nc.dram_tensor(name, shape, dty, kind="ExternalInput")
    tout = nc.dram_tensor("tout", [128, B], dt, kind="ExternalOutput")

    with tile.TileContext(nc) as tc:
        with tc.tile_pool(name="const", bufs=1) as cpool:
            params = cpool.tile([128, 6 * MT], dt)
            s1 = params[:, 0:MT]
            b1 = params[:, MT:2 * MT]
            s2 = params[:, 2 * MT:3 * MT]
            b2 = params[:, 3 * MT:4 * MT]
            c0 = params[:, 4 * MT:5 * MT]
            sg = params[:, 5 * MT:6 * MT]
            zst = cpool.tile([128, B], bfdt)
            dstb = cpool.tile([128, B], bfdt)
            rhsc = cpool.tile([2, B], f32r)
            that_all = cpool.tile([128, MT * 128], bfdt)
            f3db_all = cpool.tile([128, ML], bfdt)
            f3c_all = cpool.tile([2, ML], f32r)
            warm = cpool.tile([128, 8], dt)

            nc.sync.dma_start(params[:, :], dram["params"][:, :])

            with (
                tc.tile_pool(name="lhs", bufs=4) as lpool,
                tc.tile_pool(name="work", bufs=2) as wpool,
                tc.tile_pool(name="etile", bufs=2) as epool,
                tc.tile_pool(name="wgtp", bufs=1) as gpool,
            ):
                pks = [lpool.tile([128, 384], bfdt, tag="pk",
                                  name=f"pk{j}") for j in range(MT)]
                QB = B // 4
                nc.sync.dma_start(pks[0][:, :], dram["pk"][:, 0:384])
                nc.scalar.dma_start(zst[:, 0:QB], dram["zst"][:, 0:QB])
                # table-load after the critical DMA trigger, before first erf
                nc.scalar.activation(warm[:, :], params[:, 0:8], AF.Erf)
                nc.gpsimd.dma_start(zst[:, QB:2 * QB],
                                    dram["zst"][:, QB:2 * QB])
                nc.scalar.dma_start(zst[:, 2 * QB:3 * QB],
                                    dram["zst"][:, 2 * QB:3 * QB])
                nc.sync.dma_start(zst[:, 3 * QB:B],
                                  dram["zst"][:, 3 * QB:B])
                nc.scalar.dma_start(rhsc[:, :], dram["rhsc"][:, :])
                nc.sync.dma_start(dstb[:, 0:HB], dram["dstb"][:, 0:HB])
                nc.scalar.dma_start(dstb[:, HB:B], dram["dstb"][:, HB:B])
                nc.scalar.dma_start(f3c_all[:, :], dram["f3c"][:, :])
                nc.sync.dma_start(f3db_all[:, :], dram["f3db"][:, :])
                for j in range(1, MT):
                    nc.sync.dma_start(pks[j][:, :],
                                      dram["pk"][:, j * 384:(j + 1) * 384])
                nc.sync.dma_start(
                    that_all[:, :].rearrange("p (j c) -> p j c", j=MT),
                    dram["that2"][:, :].rearrange("(j p) c -> p j c", p=128))

                wgts = []
                with (
                    tc.tile_pool(name="xps", bufs=1, space="PSUM") as xpool,
                    tc.tile_pool(name="yps", bufs=1, space="PSUM") as ypool,
                    tc.tile_pool(name="bps", bufs=2, space="PSUM") as bpool,
                ):
                    pending = None

                    def finalize(st):
                        bs2_p, EB_p, wgt_p, hh_p, hs_p, j_p = st
                        nc.scalar.activation(EB_p[:, :], bs2_p[:, :],
                                             AF.Sigmoid,
                                             bias=c0[:, j_p:j_p + 1])
                        nc.vector.tensor_mul(wgt_p[:, hs_p], EB_p[:, :],
                                             hh_p[:, hs_p])

                    for j in range(MT):
                        pk_t = pks[j]
                        djx_t = pk_t[:, 0:128]
                        djy_t = pk_t[:, 128:256]
                        f3z_t = pk_t[:, 256:384]
                        f3db_t = f3db_all[:, j * 128:(j + 1) * 128]
                        f3c_t = f3c_all[:, j * 128:(j + 1) * 128]

                        wgt = gpool.tile([128, B], bfdt, tag=f"wgt{j}")
                        wgts.append(wgt)
                        # per-tile full-B e1/e2 so h is one wide op
                        e1 = epool.tile([128, B], dt, tag="e1")
                        e2 = epool.tile([128, B], dt, tag="e2")
                        hh = epool.tile([128, B], bfdt, tag="hh")
                        for h in range(NH):
                            hs = slice(h * HB, (h + 1) * HB)
                            x_ps = xpool.tile([128, HB], dt, tag="x",
                                              name=f"x{j}_{h}")
                            y_ps = ypool.tile([128, HB], dt, tag="y",
                                              name=f"y{j}_{h}")
                            b_ps = bpool.tile([128, HB], dt, tag="b",
                                              name=f"b{j}_{h}")
                            xx = wpool.tile([128, HB], dt, tag="xx")
                            ysb = wpool.tile([128, HB], dt, tag="ysb")
                            ysq = wpool.tile([128, HB], dt, tag="ysq")
                            bs1 = wpool.tile([128, HB], dt, tag="bs1")
                            bs2 = wpool.tile([128, HB], dt, tag="bs2")
                            EB = wpool.tile([128, HB], bfdt, tag="EB")

                            for q in range(2):
                                qs = slice(h * HB + q * 512,
                                           h * HB + (q + 1) * 512)
                                qo = slice(q * 512, (q + 1) * 512)
                                nc.tensor.matmul(x_ps[:, qo], djx_t,
                                                 zst[:, qs],
                                                 start=True, stop=True)
                            for q in range(2):
                                qs = slice(h * HB + q * 512,
                                           h * HB + (q + 1) * 512)
                                qo = slice(q * 512, (q + 1) * 512)
                                nc.tensor.matmul(y_ps[:, qo], djy_t,
                                                 zst[:, qs],
                                                 start=True, stop=True)
                            nc.vector.tensor_copy(ysb[:, :], y_ps[:, :])
                            nc.gpsimd.tensor_mul(ysq[:, :], ysb[:, :],
                                                 ysb[:, :])
                            # ACT erf pair + x-square off x_ps
                            nc.scalar.activation(e1[:, hs], x_ps[:, :],
                                                 AF.Erf,
                                                 bias=b1[:, j:j + 1],
                                                 scale=s1[:, j:j + 1])
                            nc.scalar.activation(e2[:, hs], x_ps[:, :],
                                                 AF.Erf,
                                                 bias=b2[:, j:j + 1],
                                                 scale=s2[:, j:j + 1])
                            nc.scalar.activation(xx[:, :], x_ps[:, :],
                                                 AF.Square)
                            nc.gpsimd.tensor_add(hh[:, hs], e1[:, hs],
                                                 e2[:, hs])
                            # base accumulation in PSUM via PE
                            for q in range(2):
                                qs = slice(h * HB + q * 512,
                                           h * HB + (q + 1) * 512)
                                qo = slice(q * 512, (q + 1) * 512)
                                nc.tensor.matmul(b_ps[:, qo], f3z_t,
                                                 zst[:, qs],
                                                 start=True, stop=False)
                                nc.tensor.matmul(b_ps[:, qo], f3db_t,
                                                 dstb[:, qs],
                                                 start=False, stop=False)
                                nc.tensor.matmul(b_ps[:, qo], f3c_t,
                                                 rhsc[:, qs],
                                                 start=False, stop=True)
                            # base = (b_ps + xx) + sgn*ysq on DVE
                            nc.vector.tensor_add(bs1[:, :], xx[:, :],
                                                 b_ps[:, :])
                            nc.vector.scalar_tensor_tensor(
                                bs2[:, :], ysq[:, :], sg[:, j:j + 1],
                                bs1[:, :], AO.mult, AO.add)
                            if pending is not None:
                                finalize(pending)
                            pending = (bs2, EB, wgt, hh, hs, j)
                    finalize(pending)

                # payload matmuls: T = sum_j that_j^T wgt_j
                with tc.tile_pool(name="tp", bufs=1, space="PSUM") as tpool:
                    ocp = wpool.tile([128, B], dt, tag="ocp")
                    tph = tpool.tile([128, B], dt, tag="tph")
                    for j in range(MT):
                        that_j = that_all[:, j * 128:(j + 1) * 128]
                        for q in range(4):
                            qo = slice(q * 512, (q + 1) * 512)
                            nc.tensor.matmul(tph[:, qo], that_j,
                                             wgts[j][:, qo],
                                             start=(j == 0),
                                             stop=(j == MT - 1))
                    for q in range(4):
                        qo = slice(q * 512, (q + 1) * 512)
                        if q % 2 == 0:
                            nc.scalar.copy(ocp[:, qo], tph[:, qo])
                        else:
                            nc.vector.tensor_copy(ocp[:, qo], tph[:, qo])
                        nc.sync.dma_start(tout[:, qo], ocp[:, qo])

    nc.compile()
    return nc


def kernel(z_re, z_im, d_re, d_im, zj_re, zj_im, dj_re, dj_im,
           That_re, That_im, alpha, sig_par, sig_perp, _emulate=False):
    p = _prep(z_re, z_im, d_re, d_im, zj_re, zj_im, dj_re, dj_im,
              That_re, That_im, alpha, sig_par, sig_perp)
    maps = [_core_slices(p, c) for c in range(NCORES)]
    dev_maps = _device_maps(maps)

    if _emulate:
        outs = [_emulate_core(m) for m in dev_maps]
    else:
        from concourse.bass_utils import run_bass_kernel_spmd
        if "nc" not in _CACHE:
            _CACHE["nc"] = _build_bass()
        res = run_bass_kernel_spmd(_CACHE["nc"], dev_maps,
                                   core_ids=list(range(NCORES)))
        outs = [res.results[c]["tout"] for c in range(NCORES)]

    full = np.zeros((128, B), np.float64)
    for o in outs:
        full += o.astype(np.float64)
    full = full.astype(f32).T                   # [B, 128]
    return (full[:, :S] + 1j * full[:, S:]).astype(np.complex64)
